# revision 23
# baseline (speedup 1.0000x reference)
"""Trainium2 Bass kernel for nn_CoordsToNRF.

out[b, p] = atom_nc[b, p] * (AU2KCALMOLA / MAX_NRF) / ||coords[b, I[p]] - coords[b, J[p]]||^2

Strategy (pure data parallel over batch, 8 cores x 128 batches):
  - Layout: batch on partitions, pairs on the free dim ([128, 8128] per core).
  - Pair gather+subtract on the TensorEngine: per xyz component,
        D_c = CT_c.T @ S
    with S [atom, pairs] the static +1/-1 tril selection matrix. fp16 matmuls
    (1 cyc/row) with a TWO-term split (~22 bit coords):
        C = C0 + C1,  C0 = fp16(C), C1 = fp16(C - C0)   (C1 may be subnormal;
    verified exact on HW). Both terms stream the SAME smat rhs; accumulate in
    PSUM f32. Coords are uploaded pre-transposed ([atom, comp, batch]) so no
    on-chip transposes are needed.
  - PSUM tile [128, 3, 512] (3 banks) per group, double buffered.
  - ScalarE: ONE activation Square per group over all 3 planes with folded
    scale s = 1/sqrt(K), bf16 out. This is the pipeline rate limiter
    (~1.53us/group).
  - VectorE at 1024-wide 2-group macros: bf16 add (2x mode), bf16 add (2x),
    fast reciprocal (bf16 in, f32 out), f32 multiply by atom_nc; half the
    multiplies go to GpSimd.
  - ~3.4us of dummy warmup matmuls at t~7us lift the PE HAM clock gate to
    2.4 GHz before the real matmuls start; input DMAs are issued in parallel
    from four engine queues with a small first smat chunk so the PE pipeline
    starts ~9.5us.
"""

import sys

for _p in ("/opt/trn_rl_repo",):
    if _p not in sys.path:
        sys.path.insert(0, _p)

import numpy as np
from contextlib import ExitStack

import concourse.bass as bass
import concourse.bacc as bacc
import concourse.tile as tile
from concourse import mybir
from concourse.bass_utils import run_bass_kernel_spmd
from concourse.dve_ops import RECIPROCAL_APPROX_FAST, RECIP_APPROX_FAST_CONSTS

F32 = mybir.dt.float32
F16 = mybir.dt.float16
BF16 = mybir.dt.bfloat16

N_ATOMS = 128
NC2 = N_ATOMS * (N_ATOMS - 1) // 2  # 8128
BATCH = 1024
N_CORES = 8
BPC = BATCH // N_CORES  # 128 batches per core

AU2KCALMOLA = 627.5095 * 0.529177
MAX_NRF = 13036.0
K_CONST = AU2KCALMOLA / MAX_NRF
SQ_SCALE = float(1.0 / np.sqrt(K_CONST))  # fold K into the square

GROUP = 512
GROUPS = [(g, min(GROUP, NC2 - g)) for g in range(0, NC2, GROUP)]
# elementwise macro widths: small first (early vector start), wide middle
# (amortize per-op overhead), small last (short drain)
MACROS = [(0, 512)] + [(m, 1024) for m in range(512, 7680, 1024)] + [(7680, 448)]
assert sum(w for _, w in MACROS) == NC2
MACRO_W = 1024  # max macro width (tile sizing)
# smat chunks: small first chunk so the PE can start early
SMAT_CHUNKS = []
_c = 0
for w in (512, 2048, 2048, 2048, 1472):
    SMAT_CHUNKS.append((_c, w))
    _c += w
assert _c == NC2
# anc chunks aligned to macro boundaries (macros never cross a chunk)
ANC_CHUNKS = [(0, 1536), (1536, 2048), (3584, 2048), (5632, 2496)]
assert sum(w for _, w in ANC_CHUNKS) == NC2


def _anc_chunk_of(ms: int, mw: int):
    for ci, (c0, cw) in enumerate(ANC_CHUNKS):
        if c0 <= ms < c0 + cw:
            assert ms + mw <= c0 + cw
            return ci, ms - c0
    raise AssertionError

# macros whose final multiply runs on GpSimd instead of Vector (mid-stream
# only: gpsimd's mul is ~2.5x slower, so keep it off the drain)
GPSIMD_MUL_MACROS = set()
BF16_RECIP = True
N_WARMUP_MM = 10

_I, _J = np.tril_indices(N_ATOMS, -1)


def _smat_chunk_of(gs: int):
    for ci, (c0, cw) in enumerate(SMAT_CHUNKS):
        if c0 <= gs < c0 + cw:
            assert gs + min(GROUP, NC2 - gs) <= c0 + cw
            return ci, gs - c0
    raise AssertionError


def _build_smat_f16() -> np.ndarray:
    s = np.zeros((N_ATOMS, NC2), dtype=np.float16)
    p = np.arange(NC2)
    s[_I, p] = 1.0
    s[_J, p] = -1.0
    return s


def _split_coords2(coords32: np.ndarray):
    """[B, A, 3] f32 -> two fp16 terms with C ~= C0 + C1 (~22 bits)."""
    c64 = coords32.astype(np.float64)
    c0 = c64.astype(np.float16)
    c1 = (c64 - c0.astype(np.float64)).astype(np.float16)
    return c0, c1


def _build_program():
    nc = bacc.Bacc("TRN2", target_bir_lowering=False, debug=False)

    ct_d = nc.dram_tensor("ctb", [N_ATOMS, 2 * 3 * BPC], F16, kind="ExternalInput")
    smat_d = nc.dram_tensor("smat", [N_ATOMS, NC2], F16, kind="ExternalInput")
    anc_d = nc.dram_tensor("atom_nc", [BPC, NC2], BF16, kind="ExternalInput")
    out_d = nc.dram_tensor("out", [BPC, NC2], BF16, kind="ExternalOutput")

    with tile.TileContext(nc) as tc, ExitStack() as ctx:
        const = ctx.enter_context(tc.tile_pool(name="const", bufs=1))
        sqp = ctx.enter_context(tc.tile_pool(name="sqp", bufs=2))
        vwork = ctx.enter_context(tc.tile_pool(name="vwork", bufs=2))
        outp = ctx.enter_context(tc.tile_pool(name="outp", bufs=2))
        ps = ctx.enter_context(tc.tile_pool(name="ps", bufs=2, space="PSUM"))
        ps_w = ctx.enter_context(tc.tile_pool(name="ps_w", bufs=1, space="PSUM"))

        # ---- PE warmup: lift the HAM clock gate while DMAs are in flight ----
        dummy = const.tile([128, 256], F16)
        nc.gpsimd.memset(dummy[:], 0.0)
        wps = ps_w.tile([128, 256], F32)
        for _ in range(N_WARMUP_MM):
            nc.tensor.matmul(
                wps[:], dummy[:, :128], dummy[:], start=True, stop=True,
                skip_group_check=True,
            )

        # ---- input DMAs, spread across engine queues so issue is parallel ----
        smat_sb = [None] * len(SMAT_CHUNKS)
        anc_sb = [None] * len(ANC_CHUNKS)
        for ci, (c0, cw) in enumerate(SMAT_CHUNKS):
            smat_sb[ci] = const.tile([N_ATOMS, cw], F16, name=f"smat{ci}")
        for ci, (c0, cw) in enumerate(ANC_CHUNKS):
            anc_sb[ci] = const.tile([BPC, cw], BF16, name=f"anc{ci}")
        ct_sb = const.tile([N_ATOMS, 2, 3, BPC], F16, name="ctb")

        # gating order: ct0/ct1 (tiny, gate all matmuls) then smat chunk0,
        # all on sync; anc chunk0 on the scalar queue; the rest on sync.
        nc.gpsimd.dma_start(
            ct_sb[:], ct_d[:, :].rearrange("a (t c b) -> a t c b", t=2, c=3))
        c0, cw = SMAT_CHUNKS[0]
        nc.gpsimd.dma_start(smat_sb[0][:], smat_d[:, c0:c0 + cw])
        c0, cw = ANC_CHUNKS[0]
        nc.scalar.dma_start(anc_sb[0][:], anc_d[:, c0:c0 + cw])
        for ci, (c0, cw) in list(enumerate(SMAT_CHUNKS))[1:]:
            nc.sync.dma_start(smat_sb[ci][:], smat_d[:, c0:c0 + cw])
        for ci, (c0, cw) in list(enumerate(ANC_CHUNKS))[1:]:
            nc.sync.dma_start(anc_sb[ci][:], anc_d[:, c0:c0 + cw])

        # ---- main loop: 2-group macros ----
        for mi, (ms, mw) in enumerate(MACROS):
            gpair = [(gs, fd) for gs, fd in GROUPS if ms <= gs < ms + mw]
            sq = sqp.tile([128, 3, MACRO_W], BF16, tag="sq")
            for gi, (gs, fd) in enumerate(gpair):
                sci, soff = _smat_chunk_of(gs)
                rhs = smat_sb[sci][:, soff:soff + fd]
                p_t = ps.tile([128, 3, GROUP], F32, tag="d")
                for c in range(3):
                    for t in range(2):
                        nc.tensor.matmul(
                            p_t[:, c, :fd], ct_sb[:, t, c, :], rhs,
                            start=(t == 0), stop=(t == 1),
                            skip_group_check=True,
                        )
                o0 = gi * GROUP
                nc.scalar.activation(
                    sq[:, :, o0:o0 + fd], p_t[:, :, :fd],
                    mybir.ActivationFunctionType.Square,
                    bias=0.0, scale=SQ_SCALE,
                )
            t12 = vwork.tile([128, MACRO_W], BF16, tag="t12")
            nc.vector.tensor_add(t12[:, :mw], sq[:, 0, :mw], sq[:, 1, :mw])
            r2h = vwork.tile([128, MACRO_W], BF16, tag="r2h")
            nc.vector.tensor_add(r2h[:, :mw], t12[:, :mw], sq[:, 2, :mw])
            o = outp.tile([128, MACRO_W], BF16)
            aci, aoff = _anc_chunk_of(ms, mw)
            anc_sl = anc_sb[aci][:, aoff:aoff + mw]
            inv = vwork.tile([128, MACRO_W], BF16, tag="inv")
            cst = RECIP_APPROX_FAST_CONSTS
            nc.vector._custom_dve(
                RECIPROCAL_APPROX_FAST, out=inv[:, :mw], in0=r2h[:, :mw],
                s0=cst["s0"], s1=cst["s1"], imm2=cst["imm2"],
            )
            mul_eng = nc.gpsimd if mi in GPSIMD_MUL_MACROS else nc.vector
            mul_eng.tensor_mul(o[:, :mw], inv[:, :mw], anc_sl)
            nc.sync.dma_start(out_d[:, ms:ms + mw], o[:, :mw])

    nc.compile()
    return nc


_CACHED = None


def _get_program():
    global _CACHED
    if _CACHED is None:
        _CACHED = _build_program()
    return _CACHED


def kernel(coords, atom_nc, _trace=False, _trace_kwargs=None):
    coords = np.ascontiguousarray(np.asarray(coords, dtype=np.float32))
    atom_nc = np.ascontiguousarray(np.asarray(atom_nc, dtype=np.float32))
    assert coords.shape == (BATCH, N_ATOMS, 3)
    assert atom_nc.shape == (BATCH, NC2)

    import ml_dtypes
    nc = _get_program()
    smat = _build_smat_f16()
    c0, c1 = _split_coords2(coords)
    anc_bf = atom_nc.astype(ml_dtypes.bfloat16)

    in_maps = []
    for core in range(N_CORES):
        b0 = core * BPC
        # [b, a, c] slice -> [a, t, c, b] pre-transposed weights, both terms
        ctb = np.stack([
            c0[b0:b0 + BPC].transpose(1, 2, 0),
            c1[b0:b0 + BPC].transpose(1, 2, 0),
        ], axis=1).reshape(N_ATOMS, 2 * 3 * BPC)
        in_maps.append({
            "ctb": np.ascontiguousarray(ctb),
            "atom_nc": anc_bf[b0:b0 + BPC],
            "smat": smat,
        })

    kw = {}
    if _trace:
        kw["trace"] = True
        kw.update(_trace_kwargs or {})
    res = run_bass_kernel_spmd(nc, in_maps, core_ids=list(range(N_CORES)), **kw)
    out = np.concatenate(
        [np.asarray(r["out"]).astype(np.float32) for r in res.results], axis=0)
    if _trace:
        return out, res
    return out


if __name__ == "__main__":
    rng = np.random.default_rng(0)
    coords = (rng.standard_normal((BATCH, N_ATOMS, 3)) * 5.0).astype(np.float32)
    atom_nc = rng.uniform(1.0, 50.0, (BATCH, NC2)).astype(np.float32)
    out = kernel(coords, atom_nc)
    print(out.shape, out.dtype)


# revision 24
# speedup vs baseline: 1.0951x; 1.0951x over previous
"""Trainium2 Bass kernel for nn_CoordsToNRF.

out[b, p] = atom_nc[b, p] * (AU2KCALMOLA / MAX_NRF) / ||coords[b, I[p]] - coords[b, J[p]]||^2

Strategy (pure data parallel over batch, 8 cores x 128 batches):
  - Layout: batch on partitions, pairs on the free dim ([128, 8128] per core).
  - Pair gather+subtract on the TensorEngine: per xyz component,
        D_c = CT_c.T @ S
    with S [atom, pairs] the static +1/-1 tril selection matrix. fp16 matmuls
    (1 cyc/row) with a TWO-term split (~22 bit coords):
        C = C0 + C1,  C0 = fp16(C), C1 = fp16(C - C0)   (C1 may be subnormal;
    verified exact on HW). Both terms stream the SAME smat rhs; accumulate in
    PSUM f32. Coords are uploaded pre-transposed ([atom, comp, batch]) so no
    on-chip transposes are needed.
  - PSUM tile [128, 3, 512] (3 banks) per group, double buffered.
  - ScalarE: ONE activation Square per group over all 3 planes with folded
    scale s = 1/sqrt(K), bf16 out. This is the pipeline rate limiter
    (~1.53us/group).
  - VectorE at 1024-wide 2-group macros: bf16 add (2x mode), bf16 add (2x),
    fast reciprocal (bf16 in, f32 out), f32 multiply by atom_nc; half the
    multiplies go to GpSimd.
  - ~3.4us of dummy warmup matmuls at t~7us lift the PE HAM clock gate to
    2.4 GHz before the real matmuls start; input DMAs are issued in parallel
    from four engine queues with a small first smat chunk so the PE pipeline
    starts ~9.5us.
"""

import sys

for _p in ("/opt/trn_rl_repo",):
    if _p not in sys.path:
        sys.path.insert(0, _p)

import numpy as np
from contextlib import ExitStack

import concourse.bass as bass
import concourse.bacc as bacc
import concourse.tile as tile
from concourse import mybir
from concourse.bass_utils import run_bass_kernel_spmd
from concourse.dve_ops import RECIPROCAL_APPROX_FAST, RECIP_APPROX_FAST_CONSTS

F32 = mybir.dt.float32
F16 = mybir.dt.float16
BF16 = mybir.dt.bfloat16

N_ATOMS = 128
NC2 = N_ATOMS * (N_ATOMS - 1) // 2  # 8128
BATCH = 1024
N_CORES = 8
BPC = BATCH // N_CORES  # 128 batches per core

AU2KCALMOLA = 627.5095 * 0.529177
MAX_NRF = 13036.0
K_CONST = AU2KCALMOLA / MAX_NRF
SQ_SCALE = float(1.0 / np.sqrt(K_CONST))  # fold K into the square

GROUP = 512
GROUPS = [(g, min(GROUP, NC2 - g)) for g in range(0, NC2, GROUP)]
# elementwise macro widths: small first (early vector start), wide middle
# (amortize per-op overhead), small last (short drain)
MACROS = [(0, 512)] + [(m, 1024) for m in range(512, 7680, 1024)] + [(7680, 448)]
assert sum(w for _, w in MACROS) == NC2
MACRO_W = 1024  # max macro width (tile sizing)
# smat chunks: small first chunk so the PE can start early
SMAT_CHUNKS = []
_c = 0
for w in (512, 2048, 2048, 2048, 1472):
    SMAT_CHUNKS.append((_c, w))
    _c += w
assert _c == NC2
# anc chunks aligned to macro boundaries (macros never cross a chunk)
ANC_CHUNKS = [(0, 1536), (1536, 2048), (3584, 2048), (5632, 2496)]
assert sum(w for _, w in ANC_CHUNKS) == NC2


def _anc_chunk_of(ms: int, mw: int):
    for ci, (c0, cw) in enumerate(ANC_CHUNKS):
        if c0 <= ms < c0 + cw:
            assert ms + mw <= c0 + cw
            return ci, ms - c0
    raise AssertionError

# macros whose final multiply runs on GpSimd instead of Vector (mid-stream
# only: gpsimd's mul is ~2.5x slower, so keep it off the drain)
GPSIMD_MUL_MACROS = set()
BF16_RECIP = True
N_WARMUP_MM = 26

_I, _J = np.tril_indices(N_ATOMS, -1)


def _smat_chunk_of(gs: int):
    for ci, (c0, cw) in enumerate(SMAT_CHUNKS):
        if c0 <= gs < c0 + cw:
            assert gs + min(GROUP, NC2 - gs) <= c0 + cw
            return ci, gs - c0
    raise AssertionError


def _build_smat_f16() -> np.ndarray:
    s = np.zeros((N_ATOMS, NC2), dtype=np.float16)
    p = np.arange(NC2)
    s[_I, p] = 1.0
    s[_J, p] = -1.0
    return s


def _split_coords2(coords32: np.ndarray):
    """[B, A, 3] f32 -> two fp16 terms with C ~= C0 + C1 (~22 bits)."""
    c64 = coords32.astype(np.float64)
    c0 = c64.astype(np.float16)
    c1 = (c64 - c0.astype(np.float64)).astype(np.float16)
    return c0, c1


def _build_program():
    nc = bacc.Bacc("TRN2", target_bir_lowering=False, debug=False)

    ct_d = nc.dram_tensor("ctb", [N_ATOMS, 2 * 3 * BPC], F16, kind="ExternalInput")
    smat_d = nc.dram_tensor("smat", [N_ATOMS, NC2], F16, kind="ExternalInput")
    anc_d = nc.dram_tensor("atom_nc", [BPC, NC2], BF16, kind="ExternalInput")
    out_d = nc.dram_tensor("out", [BPC, NC2], BF16, kind="ExternalOutput")

    with tile.TileContext(nc) as tc, ExitStack() as ctx:
        const = ctx.enter_context(tc.tile_pool(name="const", bufs=1))
        sqp = ctx.enter_context(tc.tile_pool(name="sqp", bufs=2))
        vwork = ctx.enter_context(tc.tile_pool(name="vwork", bufs=2))
        outp = ctx.enter_context(tc.tile_pool(name="outp", bufs=2))
        ps = ctx.enter_context(tc.tile_pool(name="ps", bufs=2, space="PSUM"))
        ps_w = ctx.enter_context(tc.tile_pool(name="ps_w", bufs=1, space="PSUM"))

        # ---- PE warmup: lift the HAM clock gate while DMAs are in flight ----
        dummy = const.tile([128, 256], F16)
        nc.gpsimd.memset(dummy[:], 0.0)
        wps = ps_w.tile([128, 256], F32)
        for _ in range(N_WARMUP_MM):
            nc.tensor.matmul(
                wps[:], dummy[:, :128], dummy[:], start=True, stop=True,
                skip_group_check=True,
            )

        # ---- input DMAs, spread across engine queues so issue is parallel ----
        smat_sb = [None] * len(SMAT_CHUNKS)
        anc_sb = [None] * len(ANC_CHUNKS)
        for ci, (c0, cw) in enumerate(SMAT_CHUNKS):
            smat_sb[ci] = const.tile([N_ATOMS, cw], F16, name=f"smat{ci}")
        for ci, (c0, cw) in enumerate(ANC_CHUNKS):
            anc_sb[ci] = const.tile([BPC, cw], BF16, name=f"anc{ci}")
        ct_sb = const.tile([N_ATOMS, 2, 3, BPC], F16, name="ctb")

        # gating order: ct0/ct1 (tiny, gate all matmuls) then smat chunk0,
        # all on sync; anc chunk0 on the scalar queue; the rest on sync.
        nc.sync.dma_start(
            ct_sb[:], ct_d[:, :].rearrange("a (t c b) -> a t c b", t=2, c=3))
        c0, cw = SMAT_CHUNKS[0]
        nc.sync.dma_start(smat_sb[0][:], smat_d[:, c0:c0 + cw])
        c0, cw = ANC_CHUNKS[0]
        nc.scalar.dma_start(anc_sb[0][:], anc_d[:, c0:c0 + cw])
        for ci, (c0, cw) in list(enumerate(SMAT_CHUNKS))[1:]:
            nc.sync.dma_start(smat_sb[ci][:], smat_d[:, c0:c0 + cw])
        for ci, (c0, cw) in list(enumerate(ANC_CHUNKS))[1:]:
            nc.sync.dma_start(anc_sb[ci][:], anc_d[:, c0:c0 + cw])

        # ---- main loop: 2-group macros ----
        for mi, (ms, mw) in enumerate(MACROS):
            gpair = [(gs, fd) for gs, fd in GROUPS if ms <= gs < ms + mw]
            sq = sqp.tile([128, 3, MACRO_W], BF16, tag="sq")
            for gi, (gs, fd) in enumerate(gpair):
                sci, soff = _smat_chunk_of(gs)
                rhs = smat_sb[sci][:, soff:soff + fd]
                p_t = ps.tile([128, 3, GROUP], F32, tag="d")
                for c in range(3):
                    for t in range(2):
                        nc.tensor.matmul(
                            p_t[:, c, :fd], ct_sb[:, t, c, :], rhs,
                            start=(t == 0), stop=(t == 1),
                            skip_group_check=True,
                        )
                o0 = gi * GROUP
                nc.scalar.activation(
                    sq[:, :, o0:o0 + fd], p_t[:, :, :fd],
                    mybir.ActivationFunctionType.Square,
                    bias=0.0, scale=SQ_SCALE,
                )
            t12 = vwork.tile([128, MACRO_W], BF16, tag="t12")
            nc.vector.tensor_add(t12[:, :mw], sq[:, 0, :mw], sq[:, 1, :mw])
            r2h = vwork.tile([128, MACRO_W], BF16, tag="r2h")
            nc.vector.tensor_add(r2h[:, :mw], t12[:, :mw], sq[:, 2, :mw])
            o = outp.tile([128, MACRO_W], BF16)
            aci, aoff = _anc_chunk_of(ms, mw)
            anc_sl = anc_sb[aci][:, aoff:aoff + mw]
            inv = vwork.tile([128, MACRO_W], BF16, tag="inv")
            cst = RECIP_APPROX_FAST_CONSTS
            nc.vector._custom_dve(
                RECIPROCAL_APPROX_FAST, out=inv[:, :mw], in0=r2h[:, :mw],
                s0=cst["s0"], s1=cst["s1"], imm2=cst["imm2"],
            )
            mul_eng = nc.gpsimd if mi in GPSIMD_MUL_MACROS else nc.vector
            mul_eng.tensor_mul(o[:, :mw], inv[:, :mw], anc_sl)
            nc.sync.dma_start(out_d[:, ms:ms + mw], o[:, :mw])

    nc.compile()
    return nc


_CACHED = None


def _get_program():
    global _CACHED
    if _CACHED is None:
        _CACHED = _build_program()
    return _CACHED


def kernel(coords, atom_nc, _trace=False, _trace_kwargs=None):
    coords = np.ascontiguousarray(np.asarray(coords, dtype=np.float32))
    atom_nc = np.ascontiguousarray(np.asarray(atom_nc, dtype=np.float32))
    assert coords.shape == (BATCH, N_ATOMS, 3)
    assert atom_nc.shape == (BATCH, NC2)

    import ml_dtypes
    nc = _get_program()
    smat = _build_smat_f16()
    c0, c1 = _split_coords2(coords)
    anc_bf = atom_nc.astype(ml_dtypes.bfloat16)

    in_maps = []
    for core in range(N_CORES):
        b0 = core * BPC
        # [b, a, c] slice -> [a, t, c, b] pre-transposed weights, both terms
        ctb = np.stack([
            c0[b0:b0 + BPC].transpose(1, 2, 0),
            c1[b0:b0 + BPC].transpose(1, 2, 0),
        ], axis=1).reshape(N_ATOMS, 2 * 3 * BPC)
        in_maps.append({
            "ctb": np.ascontiguousarray(ctb),
            "atom_nc": anc_bf[b0:b0 + BPC],
            "smat": smat,
        })

    kw = {}
    if _trace:
        kw["trace"] = True
        kw.update(_trace_kwargs or {})
    res = run_bass_kernel_spmd(nc, in_maps, core_ids=list(range(N_CORES)), **kw)
    out = np.concatenate(
        [np.asarray(r["out"]).astype(np.float32) for r in res.results], axis=0)
    if _trace:
        return out, res
    return out


if __name__ == "__main__":
    rng = np.random.default_rng(0)
    coords = (rng.standard_normal((BATCH, N_ATOMS, 3)) * 5.0).astype(np.float32)
    atom_nc = rng.uniform(1.0, 50.0, (BATCH, NC2)).astype(np.float32)
    out = kernel(coords, atom_nc)
    print(out.shape, out.dtype)


# revision 25
# speedup vs baseline: 1.1030x; 1.0072x over previous
"""Trainium2 Bass kernel for nn_CoordsToNRF.

out[b, p] = atom_nc[b, p] * (AU2KCALMOLA / MAX_NRF) / ||coords[b, I[p]] - coords[b, J[p]]||^2

Strategy (pure data parallel over batch, 8 cores x 128 batches):
  - Layout: batch on partitions, pairs on the free dim ([128, 8128] per core).
  - Pair gather+subtract on the TensorEngine: per xyz component,
        D_c = CT_c.T @ S
    with S [atom, pairs] the static +1/-1 tril selection matrix. fp16 matmuls
    (1 cyc/row) with a TWO-term split (~22 bit coords):
        C = C0 + C1,  C0 = fp16(C), C1 = fp16(C - C0)   (C1 may be subnormal;
    verified exact on HW). Both terms stream the SAME smat rhs; accumulate in
    PSUM f32. Coords are uploaded pre-transposed ([atom, comp, batch]) so no
    on-chip transposes are needed.
  - PSUM tile [128, 3, 512] (3 banks) per group, double buffered.
  - ScalarE: ONE activation Square per group over all 3 planes with folded
    scale s = 1/sqrt(K), bf16 out. This is the pipeline rate limiter
    (~1.53us/group).
  - VectorE at 1024-wide 2-group macros: bf16 add (2x mode), bf16 add (2x),
    fast reciprocal (bf16 in, f32 out), f32 multiply by atom_nc; half the
    multiplies go to GpSimd.
  - ~3.4us of dummy warmup matmuls at t~7us lift the PE HAM clock gate to
    2.4 GHz before the real matmuls start; input DMAs are issued in parallel
    from four engine queues with a small first smat chunk so the PE pipeline
    starts ~9.5us.
"""

import sys

for _p in ("/opt/trn_rl_repo",):
    if _p not in sys.path:
        sys.path.insert(0, _p)

import numpy as np
from contextlib import ExitStack

import concourse.bass as bass
import concourse.bacc as bacc
import concourse.tile as tile
from concourse import mybir
from concourse.bass_utils import run_bass_kernel_spmd
from concourse.dve_ops import RECIPROCAL_APPROX_FAST, RECIP_APPROX_FAST_CONSTS

F32 = mybir.dt.float32
F16 = mybir.dt.float16
BF16 = mybir.dt.bfloat16

N_ATOMS = 128
NC2 = N_ATOMS * (N_ATOMS - 1) // 2  # 8128
BATCH = 1024
N_CORES = 8
BPC = BATCH // N_CORES  # 128 batches per core

AU2KCALMOLA = 627.5095 * 0.529177
MAX_NRF = 13036.0
K_CONST = AU2KCALMOLA / MAX_NRF
SQ_SCALE = float(1.0 / np.sqrt(K_CONST))  # fold K into the square

GROUP = 512
GROUPS = [(g, min(GROUP, NC2 - g)) for g in range(0, NC2, GROUP)]
# elementwise macro widths: small first (early vector start), wide middle
# (amortize per-op overhead), small last (short drain)
MACROS = [(0, 512)] + [(m, 1024) for m in range(512, 7680, 1024)] + [(7680, 448)]
assert sum(w for _, w in MACROS) == NC2
MACRO_W = 1024  # max macro width (tile sizing)
# smat chunks: small first chunk so the PE can start early
SMAT_CHUNKS = []
_c = 0
for w in (512, 2048, 2048, 2048, 1472):
    SMAT_CHUNKS.append((_c, w))
    _c += w
assert _c == NC2
# anc chunks aligned to macro boundaries (macros never cross a chunk)
ANC_CHUNKS = [(0, 1536), (1536, 2048), (3584, 2048), (5632, 2496)]
assert sum(w for _, w in ANC_CHUNKS) == NC2


def _anc_chunk_of(ms: int, mw: int):
    for ci, (c0, cw) in enumerate(ANC_CHUNKS):
        if c0 <= ms < c0 + cw:
            assert ms + mw <= c0 + cw
            return ci, ms - c0
    raise AssertionError

# macros whose final multiply runs on GpSimd instead of Vector (mid-stream
# only: gpsimd's mul is ~2.5x slower, so keep it off the drain)
GPSIMD_MUL_MACROS = set()
BF16_RECIP = True
N_WARMUP_MM = 26

_I, _J = np.tril_indices(N_ATOMS, -1)


def _smat_chunk_of(gs: int):
    for ci, (c0, cw) in enumerate(SMAT_CHUNKS):
        if c0 <= gs < c0 + cw:
            assert gs + min(GROUP, NC2 - gs) <= c0 + cw
            return ci, gs - c0
    raise AssertionError


def _build_smat_f16() -> np.ndarray:
    s = np.zeros((N_ATOMS, NC2), dtype=np.float16)
    p = np.arange(NC2)
    s[_I, p] = 1.0
    s[_J, p] = -1.0
    return s


def _split_coords2(coords32: np.ndarray):
    """[B, A, 3] f32 -> two fp16 terms with C ~= C0 + C1 (~22 bits)."""
    c64 = coords32.astype(np.float64)
    c0 = c64.astype(np.float16)
    c1 = (c64 - c0.astype(np.float64)).astype(np.float16)
    return c0, c1


def _build_program():
    nc = bacc.Bacc("TRN2", target_bir_lowering=False, debug=False)

    boot_d = nc.dram_tensor("boot", [N_ATOMS, 2 * 3 * BPC + 512], F16,
                            kind="ExternalInput")
    smat_d = nc.dram_tensor("smat", [N_ATOMS, NC2], F16, kind="ExternalInput")
    anc_d = nc.dram_tensor("atom_nc", [BPC, NC2], BF16, kind="ExternalInput")
    out_d = nc.dram_tensor("out", [BPC, NC2], BF16, kind="ExternalOutput")

    with tile.TileContext(nc) as tc, ExitStack() as ctx:
        const = ctx.enter_context(tc.tile_pool(name="const", bufs=1))
        sqp = ctx.enter_context(tc.tile_pool(name="sqp", bufs=2))
        vwork = ctx.enter_context(tc.tile_pool(name="vwork", bufs=2))
        outp = ctx.enter_context(tc.tile_pool(name="outp", bufs=2))
        ps = ctx.enter_context(tc.tile_pool(name="ps", bufs=2, space="PSUM"))
        ps_w = ctx.enter_context(tc.tile_pool(name="ps_w", bufs=1, space="PSUM"))

        # ---- PE warmup: lift the HAM clock gate while DMAs are in flight ----
        dummy = const.tile([128, 256], F16)
        nc.gpsimd.memset(dummy[:], 0.0)
        wps = ps_w.tile([128, 256], F32)
        for _ in range(N_WARMUP_MM):
            nc.tensor.matmul(
                wps[:], dummy[:, :128], dummy[:], start=True, stop=True,
                skip_group_check=True,
            )

        # ---- input DMAs, spread across engine queues so issue is parallel ----
        smat_sb = [None] * len(SMAT_CHUNKS)
        anc_sb = [None] * len(ANC_CHUNKS)
        for ci, (c0, cw) in enumerate(SMAT_CHUNKS):
            if ci == 0:
                continue  # chunk0 rides in the boot tile
            smat_sb[ci] = const.tile([N_ATOMS, cw], F16, name=f"smat{ci}")
        for ci, (c0, cw) in enumerate(ANC_CHUNKS):
            anc_sb[ci] = const.tile([BPC, cw], BF16, name=f"anc{ci}")
        boot_sb = const.tile([N_ATOMS, 2 * 3 * BPC + 512], F16, name="boot")

        # gating order: ct0/ct1 (tiny, gate all matmuls) then smat chunk0,
        # all on sync; anc chunk0 on the scalar queue; the rest on sync.
        nc.sync.dma_start(boot_sb[:], boot_d[:, :])
        c0, cw = ANC_CHUNKS[0]
        nc.scalar.dma_start(anc_sb[0][:], anc_d[:, c0:c0 + cw])
        for ci, (c0, cw) in list(enumerate(SMAT_CHUNKS))[1:]:
            nc.sync.dma_start(smat_sb[ci][:], smat_d[:, c0:c0 + cw])
        for ci, (c0, cw) in list(enumerate(ANC_CHUNKS))[1:]:
            nc.sync.dma_start(anc_sb[ci][:], anc_d[:, c0:c0 + cw])

        # ---- main loop: 2-group macros ----
        for mi, (ms, mw) in enumerate(MACROS):
            gpair = [(gs, fd) for gs, fd in GROUPS if ms <= gs < ms + mw]
            sq = sqp.tile([128, 3, MACRO_W], BF16, tag="sq")
            for gi, (gs, fd) in enumerate(gpair):
                sci, soff = _smat_chunk_of(gs)
                if sci == 0:
                    rhs = boot_sb[:, 2 * 3 * BPC + soff:2 * 3 * BPC + soff + fd]
                else:
                    rhs = smat_sb[sci][:, soff:soff + fd]
                p_t = ps.tile([128, 3, GROUP], F32, tag="d")
                for c in range(3):
                    for t in range(2):
                        nc.tensor.matmul(
                            p_t[:, c, :fd],
                            boot_sb[:, t * 384 + c * 128:t * 384 + c * 128 + 128],
                            rhs,
                            start=(t == 0), stop=(t == 1),
                            skip_group_check=True,
                        )
                o0 = gi * GROUP
                nc.scalar.activation(
                    sq[:, :, o0:o0 + fd], p_t[:, :, :fd],
                    mybir.ActivationFunctionType.Square,
                    bias=0.0, scale=SQ_SCALE,
                )
            t12 = vwork.tile([128, MACRO_W], BF16, tag="t12")
            nc.vector.tensor_add(t12[:, :mw], sq[:, 0, :mw], sq[:, 1, :mw])
            r2h = vwork.tile([128, MACRO_W], BF16, tag="r2h")
            nc.vector.tensor_add(r2h[:, :mw], t12[:, :mw], sq[:, 2, :mw])
            o = outp.tile([128, MACRO_W], BF16)
            aci, aoff = _anc_chunk_of(ms, mw)
            anc_sl = anc_sb[aci][:, aoff:aoff + mw]
            inv = vwork.tile([128, MACRO_W], BF16, tag="inv")
            cst = RECIP_APPROX_FAST_CONSTS
            nc.vector._custom_dve(
                RECIPROCAL_APPROX_FAST, out=inv[:, :mw], in0=r2h[:, :mw],
                s0=cst["s0"], s1=cst["s1"], imm2=cst["imm2"],
            )
            mul_eng = nc.gpsimd if mi in GPSIMD_MUL_MACROS else nc.vector
            mul_eng.tensor_mul(o[:, :mw], inv[:, :mw], anc_sl)
            nc.sync.dma_start(out_d[:, ms:ms + mw], o[:, :mw])

    nc.compile()
    return nc


_CACHED = None


def _get_program():
    global _CACHED
    if _CACHED is None:
        _CACHED = _build_program()
    return _CACHED


def kernel(coords, atom_nc, _trace=False, _trace_kwargs=None):
    coords = np.ascontiguousarray(np.asarray(coords, dtype=np.float32))
    atom_nc = np.ascontiguousarray(np.asarray(atom_nc, dtype=np.float32))
    assert coords.shape == (BATCH, N_ATOMS, 3)
    assert atom_nc.shape == (BATCH, NC2)

    import ml_dtypes
    nc = _get_program()
    smat = _build_smat_f16()
    c0, c1 = _split_coords2(coords)
    anc_bf = atom_nc.astype(ml_dtypes.bfloat16)

    in_maps = []
    for core in range(N_CORES):
        b0 = core * BPC
        # [b, a, c] slice -> [a, t, c, b] pre-transposed weights, both terms
        ctb = np.stack([
            c0[b0:b0 + BPC].transpose(1, 2, 0),
            c1[b0:b0 + BPC].transpose(1, 2, 0),
        ], axis=1).reshape(N_ATOMS, 2 * 3 * BPC)
        boot = np.concatenate([ctb, smat[:, :512]], axis=1)
        in_maps.append({
            "boot": np.ascontiguousarray(boot),
            "atom_nc": anc_bf[b0:b0 + BPC],
            "smat": smat,
        })

    kw = {}
    if _trace:
        kw["trace"] = True
        kw.update(_trace_kwargs or {})
    res = run_bass_kernel_spmd(nc, in_maps, core_ids=list(range(N_CORES)), **kw)
    out = np.concatenate(
        [np.asarray(r["out"]).astype(np.float32) for r in res.results], axis=0)
    if _trace:
        return out, res
    return out


if __name__ == "__main__":
    rng = np.random.default_rng(0)
    coords = (rng.standard_normal((BATCH, N_ATOMS, 3)) * 5.0).astype(np.float32)
    atom_nc = rng.uniform(1.0, 50.0, (BATCH, NC2)).astype(np.float32)
    out = kernel(coords, atom_nc)
    print(out.shape, out.dtype)


# revision 26
# speedup vs baseline: 1.1076x; 1.0042x over previous
"""Trainium2 Bass kernel for nn_CoordsToNRF.

out[b, p] = atom_nc[b, p] * (AU2KCALMOLA / MAX_NRF) / ||coords[b, I[p]] - coords[b, J[p]]||^2

Strategy (pure data parallel over batch, 8 cores x 128 batches):
  - Layout: batch on partitions, pairs on the free dim ([128, 8128] per core).
  - Pair gather+subtract on the TensorEngine: per xyz component,
        D_c = CT_c.T @ S
    with S [atom, pairs] the static +1/-1 tril selection matrix. fp16 matmuls
    (1 cyc/row) with a TWO-term split (~22 bit coords):
        C = C0 + C1,  C0 = fp16(C), C1 = fp16(C - C0)   (C1 may be subnormal;
    verified exact on HW). Both terms stream the SAME smat rhs; accumulate in
    PSUM f32. Coords are uploaded pre-transposed ([atom, comp, batch]) so no
    on-chip transposes are needed.
  - PSUM tile [128, 3, 512] (3 banks) per group, double buffered.
  - ScalarE: ONE activation Square per group over all 3 planes with folded
    scale s = 1/sqrt(K), bf16 out. This is the pipeline rate limiter
    (~1.53us/group).
  - VectorE at 1024-wide 2-group macros: bf16 add (2x mode), bf16 add (2x),
    fast reciprocal (bf16 in, f32 out), f32 multiply by atom_nc; half the
    multiplies go to GpSimd.
  - ~3.4us of dummy warmup matmuls at t~7us lift the PE HAM clock gate to
    2.4 GHz before the real matmuls start; input DMAs are issued in parallel
    from four engine queues with a small first smat chunk so the PE pipeline
    starts ~9.5us.
"""

import sys

for _p in ("/opt/trn_rl_repo",):
    if _p not in sys.path:
        sys.path.insert(0, _p)

import numpy as np
from contextlib import ExitStack

import concourse.bass as bass
import concourse.bacc as bacc
import concourse.tile as tile
from concourse import mybir
from concourse.bass_utils import run_bass_kernel_spmd
from concourse.dve_ops import RECIPROCAL_APPROX_FAST, RECIP_APPROX_FAST_CONSTS

F32 = mybir.dt.float32
F16 = mybir.dt.float16
BF16 = mybir.dt.bfloat16

N_ATOMS = 128
NC2 = N_ATOMS * (N_ATOMS - 1) // 2  # 8128
BATCH = 1024
N_CORES = 8
BPC = BATCH // N_CORES  # 128 batches per core

AU2KCALMOLA = 627.5095 * 0.529177
MAX_NRF = 13036.0
K_CONST = AU2KCALMOLA / MAX_NRF
SQ_SCALE = float(1.0 / np.sqrt(K_CONST))  # fold K into the square

GROUP = 512
GROUPS = [(g, min(GROUP, NC2 - g)) for g in range(0, NC2, GROUP)]
# elementwise macro widths: small first (early vector start), wide middle
# (amortize per-op overhead), small last (short drain)
MACROS = [(0, 512)] + [(m, 1024) for m in range(512, 7680, 1024)] + [(7680, 448)]
assert sum(w for _, w in MACROS) == NC2
MACRO_W = 1024  # max macro width (tile sizing)
# smat chunks: small first chunk so the PE can start early
SMAT_CHUNKS = []
_c = 0
for w in (512, 2048, 2048, 2048, 1472):
    SMAT_CHUNKS.append((_c, w))
    _c += w
assert _c == NC2
# anc chunks aligned to macro boundaries (macros never cross a chunk)
ANC_CHUNKS = [(0, 1536), (1536, 2048), (3584, 2048), (5632, 2496)]
assert sum(w for _, w in ANC_CHUNKS) == NC2


def _anc_chunk_of(ms: int, mw: int):
    for ci, (c0, cw) in enumerate(ANC_CHUNKS):
        if c0 <= ms < c0 + cw:
            assert ms + mw <= c0 + cw
            return ci, ms - c0
    raise AssertionError

# macros whose final multiply runs on GpSimd instead of Vector (mid-stream
# only: gpsimd's mul is ~2.5x slower, so keep it off the drain)
GPSIMD_MUL_MACROS = set()
BF16_RECIP = True
N_WARMUP_MM = 18

_I, _J = np.tril_indices(N_ATOMS, -1)


def _smat_chunk_of(gs: int):
    for ci, (c0, cw) in enumerate(SMAT_CHUNKS):
        if c0 <= gs < c0 + cw:
            assert gs + min(GROUP, NC2 - gs) <= c0 + cw
            return ci, gs - c0
    raise AssertionError


def _build_smat_f16() -> np.ndarray:
    s = np.zeros((N_ATOMS, NC2), dtype=np.float16)
    p = np.arange(NC2)
    s[_I, p] = 1.0
    s[_J, p] = -1.0
    return s


def _split_coords2(coords32: np.ndarray):
    """[B, A, 3] f32 -> two fp16 terms with C ~= C0 + C1 (~22 bits)."""
    c64 = coords32.astype(np.float64)
    c0 = c64.astype(np.float16)
    c1 = (c64 - c0.astype(np.float64)).astype(np.float16)
    return c0, c1


def _build_program():
    nc = bacc.Bacc("TRN2", target_bir_lowering=False, debug=False)

    boot_d = nc.dram_tensor("boot", [N_ATOMS, 2 * 3 * BPC + 512], F16,
                            kind="ExternalInput")
    smat_d = nc.dram_tensor("smat", [N_ATOMS, NC2], F16, kind="ExternalInput")
    anc_d = nc.dram_tensor("atom_nc", [BPC, NC2], BF16, kind="ExternalInput")
    out_d = nc.dram_tensor("out", [BPC, NC2], BF16, kind="ExternalOutput")

    with tile.TileContext(nc) as tc, ExitStack() as ctx:
        const = ctx.enter_context(tc.tile_pool(name="const", bufs=1))
        sqp = ctx.enter_context(tc.tile_pool(name="sqp", bufs=2))
        vwork = ctx.enter_context(tc.tile_pool(name="vwork", bufs=2))
        outp = ctx.enter_context(tc.tile_pool(name="outp", bufs=2))
        ps = ctx.enter_context(tc.tile_pool(name="ps", bufs=2, space="PSUM"))
        ps_w = ctx.enter_context(tc.tile_pool(name="ps_w", bufs=1, space="PSUM"))

        # ---- PE warmup: lift the HAM clock gate while DMAs are in flight ----
        dummy = const.tile([128, 256], F16)
        nc.gpsimd.memset(dummy[:], 0.0)
        wps = ps_w.tile([128, 256], F32)
        for _ in range(N_WARMUP_MM):
            nc.tensor.matmul(
                wps[:], dummy[:, :128], dummy[:], start=True, stop=True,
                skip_group_check=True,
            )

        # ---- input DMAs, spread across engine queues so issue is parallel ----
        smat_sb = [None] * len(SMAT_CHUNKS)
        anc_sb = [None] * len(ANC_CHUNKS)
        for ci, (c0, cw) in enumerate(SMAT_CHUNKS):
            if ci == 0:
                continue  # chunk0 rides in the boot tile
            smat_sb[ci] = const.tile([N_ATOMS, cw], F16, name=f"smat{ci}")
        for ci, (c0, cw) in enumerate(ANC_CHUNKS):
            anc_sb[ci] = const.tile([BPC, cw], BF16, name=f"anc{ci}")
        boot_sb = const.tile([N_ATOMS, 2 * 3 * BPC + 512], F16, name="boot")

        # gating order: ct0/ct1 (tiny, gate all matmuls) then smat chunk0,
        # all on sync; anc chunk0 on the scalar queue; the rest on sync.
        nc.sync.dma_start(boot_sb[:], boot_d[:, :])
        c0, cw = ANC_CHUNKS[0]
        nc.scalar.dma_start(anc_sb[0][:], anc_d[:, c0:c0 + cw])
        for ci, (c0, cw) in list(enumerate(SMAT_CHUNKS))[1:]:
            nc.sync.dma_start(smat_sb[ci][:], smat_d[:, c0:c0 + cw])
        for ci, (c0, cw) in list(enumerate(ANC_CHUNKS))[1:]:
            nc.sync.dma_start(anc_sb[ci][:], anc_d[:, c0:c0 + cw])

        # ---- main loop: 2-group macros ----
        for mi, (ms, mw) in enumerate(MACROS):
            gpair = [(gs, fd) for gs, fd in GROUPS if ms <= gs < ms + mw]
            sq = sqp.tile([128, 3, MACRO_W], BF16, tag="sq")
            for gi, (gs, fd) in enumerate(gpair):
                sci, soff = _smat_chunk_of(gs)
                if sci == 0:
                    rhs = boot_sb[:, 2 * 3 * BPC + soff:2 * 3 * BPC + soff + fd]
                else:
                    rhs = smat_sb[sci][:, soff:soff + fd]
                p_t = ps.tile([128, 3, GROUP], F32, tag="d")
                for c in range(3):
                    for t in range(2):
                        nc.tensor.matmul(
                            p_t[:, c, :fd],
                            boot_sb[:, t * 384 + c * 128:t * 384 + c * 128 + 128],
                            rhs,
                            start=(t == 0), stop=(t == 1),
                            skip_group_check=True,
                        )
                o0 = gi * GROUP
                nc.scalar.activation(
                    sq[:, :, o0:o0 + fd], p_t[:, :, :fd],
                    mybir.ActivationFunctionType.Square,
                    bias=0.0, scale=SQ_SCALE,
                )
            t12 = vwork.tile([128, MACRO_W], BF16, tag="t12")
            nc.vector.tensor_add(t12[:, :mw], sq[:, 0, :mw], sq[:, 1, :mw])
            r2h = vwork.tile([128, MACRO_W], BF16, tag="r2h")
            nc.vector.tensor_add(r2h[:, :mw], t12[:, :mw], sq[:, 2, :mw])
            o = outp.tile([128, MACRO_W], BF16)
            aci, aoff = _anc_chunk_of(ms, mw)
            anc_sl = anc_sb[aci][:, aoff:aoff + mw]
            inv = vwork.tile([128, MACRO_W], BF16, tag="inv")
            cst = RECIP_APPROX_FAST_CONSTS
            nc.vector._custom_dve(
                RECIPROCAL_APPROX_FAST, out=inv[:, :mw], in0=r2h[:, :mw],
                s0=cst["s0"], s1=cst["s1"], imm2=cst["imm2"],
            )
            mul_eng = nc.gpsimd if mi in GPSIMD_MUL_MACROS else nc.vector
            mul_eng.tensor_mul(o[:, :mw], inv[:, :mw], anc_sl)
            nc.sync.dma_start(out_d[:, ms:ms + mw], o[:, :mw])

    nc.compile()
    return nc


_CACHED = None


def _get_program():
    global _CACHED
    if _CACHED is None:
        _CACHED = _build_program()
    return _CACHED


def kernel(coords, atom_nc, _trace=False, _trace_kwargs=None):
    coords = np.ascontiguousarray(np.asarray(coords, dtype=np.float32))
    atom_nc = np.ascontiguousarray(np.asarray(atom_nc, dtype=np.float32))
    assert coords.shape == (BATCH, N_ATOMS, 3)
    assert atom_nc.shape == (BATCH, NC2)

    import ml_dtypes
    nc = _get_program()
    smat = _build_smat_f16()
    c0, c1 = _split_coords2(coords)
    anc_bf = atom_nc.astype(ml_dtypes.bfloat16)

    in_maps = []
    for core in range(N_CORES):
        b0 = core * BPC
        # [b, a, c] slice -> [a, t, c, b] pre-transposed weights, both terms
        ctb = np.stack([
            c0[b0:b0 + BPC].transpose(1, 2, 0),
            c1[b0:b0 + BPC].transpose(1, 2, 0),
        ], axis=1).reshape(N_ATOMS, 2 * 3 * BPC)
        boot = np.concatenate([ctb, smat[:, :512]], axis=1)
        in_maps.append({
            "boot": np.ascontiguousarray(boot),
            "atom_nc": anc_bf[b0:b0 + BPC],
            "smat": smat,
        })

    kw = {}
    if _trace:
        kw["trace"] = True
        kw.update(_trace_kwargs or {})
    res = run_bass_kernel_spmd(nc, in_maps, core_ids=list(range(N_CORES)), **kw)
    out = np.concatenate(
        [np.asarray(r["out"]).astype(np.float32) for r in res.results], axis=0)
    if _trace:
        return out, res
    return out


if __name__ == "__main__":
    rng = np.random.default_rng(0)
    coords = (rng.standard_normal((BATCH, N_ATOMS, 3)) * 5.0).astype(np.float32)
    atom_nc = rng.uniform(1.0, 50.0, (BATCH, NC2)).astype(np.float32)
    out = kernel(coords, atom_nc)
    print(out.shape, out.dtype)
